# revision 10
# baseline (speedup 1.0000x reference)
"""ColorMLP Trainium2 kernel.

Reference computation (per pixel, 8 input channels):
    h1 = relu(x @ w0 + b0)         # 8 -> 16
    h2 = relu(h1 @ w1 + b1)        # 16 -> 16
    y  = sigmoid(h2 @ w2 + b2)     # 16 -> 3
    out = mask * ((1-res)*rgb + res*y)   rgb = x[..,:3], res = x[..,3]

Strategy (pure data parallel over 8 cores, 1,048,576 px each):
  - SWDGE cast-DMA loads x as bf16 pixel-major [128, 2048] per 32768-px batch.
  - One XBAR DMA-transpose per batch (HWDGE on the idle Sync queue,
    SBUF->SBUF bf16, 3D out form = 16 chunk-transposes in one instruction)
    turns each [128,128] chunk feature-major: t [128 = 16grp x 8f, 2048].
    This keeps both the PE transposes and the PSUM-evacuation copies off
    the compute engines entirely.
  - L0: 64x128 row-tiled block-diag(w0 x8) matmuls (2 concurrent row-group
    tiles) -> h_ps [128, 1024] (two PSUM banks, one per s-half).
  - L1: full-array block-diag(w1 x8) matmul per s-half -> h2_ps [128, 1024].
  - L2 fused with the output transpose: lhsT = relu(h2) chunk [128,128],
    rhs = block-diag(w2 x8) [128,24] -> pre-sigmoid z lands PIXEL-MAJOR in
    PSUM.
  - relu evacuations run on [128,1024] two-bank PSUM views, statically
    load-balanced between ScalarE (ACT) and VectorE (DVE); sigmoid on ACT;
    blend on DVE (bit-exact with the mask folded into the coefficients).
"""

import os
import sys

import numpy as np

sys.path.insert(0, "/opt/trn_rl_repo")

import ml_dtypes

import concourse.bacc as bacc
import concourse.bass as bass
import concourse.mybir as mybir
import concourse.tile as tile
from concourse.bass_utils import run_bass_kernel_spmd

F32 = mybir.dt.float32
BF16 = mybir.dt.bfloat16
U8 = mybir.dt.uint8

N_CORES = 8
B, H, W = 8, 1024, 1024
IN_DIM, HID, OUT_DIM = 8, 16, 3
NPX = B * H * W                  # 8388608
NPC = NPX // N_CORES             # 1048576 per core

BATCH_PX = 32768                 # pixels per batch (4 supertiles)
PPP = BATCH_PX // 128            # 256 px per partition per batch
N_ST = 4                         # supertiles per batch (each 8192 px)

# Of the 16 PSUM-evacuation relu units per batch (8x relu1 + 8x relu2, each a
# [128,512] one-bank op), ACT takes relu1 for all (st,s) plus relu2 for
# (st,s) in ACT_RELU2_UNITS; DVE takes the rest. Tuned for load balance.
ACT_RELU2_UNITS = ((0, 0), (2, 0))


def _bd(w, reps):
    """Block-diagonal of `w` repeated `reps` times: [reps*K, reps*M]."""
    k, m = w.shape
    out = np.zeros((reps * k, reps * m), np.float32)
    for g in range(reps):
        out[g * k:(g + 1) * k, g * m:(g + 1) * m] = w
    return out


def _prep_weights(w0, b0, w1, b1, w2, b2):
    """Host-side constant prep. Returns dict of named numpy arrays."""
    bf = ml_dtypes.bfloat16
    # W0T [128,128]: rows 0-63 & 64-127 are each blockdiag8(w0) [64,128]
    bd0 = _bd(w0, 8)  # [64, 128]
    w0t = np.concatenate([bd0, bd0], axis=0)  # [128, 128]
    # W1BD [128,128] = blockdiag8(w1)
    w1bd = _bd(w1, 8)
    # G2 [128, 24] = blockdiag8(w2)
    g2 = _bd(w2, 8)  # [128, 24]
    # per-partition biases for h rows (g,j) -> b[j]
    b0col = np.tile(b0, 8).astype(np.float32).reshape(128, 1)
    b1col = np.tile(b1, 8).astype(np.float32).reshape(128, 1)
    # b2 pattern along free dim of z: (c*16 + s*8 + g)*3 + ch for one batch-row
    # of 192 cols per supertile -> b2 tiled 64x
    b2row = np.tile(b2, 64).astype(np.float32).reshape(1, 192)
    return {
        "W0T": w0t.astype(bf),
        "W1BD": w1bd.astype(bf),
        "G2": g2.astype(bf),
        "B0COL": b0col,
        "B1COL": b1col,
        "B2ROW": b2row,
        "b01_nonzero": bool(np.any(b0 != 0.0) or np.any(b1 != 0.0)),
        "b2_nonzero": bool(np.any(b2 != 0.0)),
    }


def build_program(npc, b01_nonzero, b2_nonzero):
    """Build the SPMD Bass program for one core processing `npc` pixels."""
    nc = bacc.Bacc("TRN2", target_bir_lowering=False, debug=False,
                   num_devices=N_CORES)
    n_batch = npc // BATCH_PX

    x_d = nc.dram_tensor("x", [npc, IN_DIM], F32, kind="ExternalInput")
    m_d = nc.dram_tensor("mask", [npc], U8, kind="ExternalInput")
    w0t_d = nc.dram_tensor("W0T", [128, 128], BF16, kind="ExternalInput")
    w1bd_d = nc.dram_tensor("W1BD", [128, 128], BF16, kind="ExternalInput")
    g2_d = nc.dram_tensor("G2", [128, 24], BF16, kind="ExternalInput")
    b0_d = nc.dram_tensor("B0COL", [128, 1], F32, kind="ExternalInput")
    b1_d = nc.dram_tensor("B1COL", [128, 1], F32, kind="ExternalInput")
    b2_d = nc.dram_tensor("B2ROW", [1, 192], F32, kind="ExternalInput")
    out_d = nc.dram_tensor("out", [npc, OUT_DIM], F32, kind="ExternalOutput")

    # DRAM views
    # batch b, partition p covers pixels b*BATCH_PX + p*PPP + [0, PPP)
    x_v = x_d[:].rearrange("(b p n) f -> b p (n f)", b=n_batch, p=128)
    m_v = m_d[:].rearrange("(b p n) -> b p n", b=n_batch, p=128)
    o_v = out_d[:].rearrange("(b p n) c -> b p (n c)", b=n_batch, p=128)

    with tile.TileContext(nc) as tc:
        with (
            tc.tile_pool(name="consts", bufs=1) as cpool,
            tc.tile_pool(name="xin", bufs=3) as xpool,
            tc.tile_pool(name="msk", bufs=3) as mpool,
            tc.tile_pool(name="tsb", bufs=3) as tpool,
            tc.tile_pool(name="hsb", bufs=3) as hpool,
            tc.tile_pool(name="h2sb", bufs=3) as h2pool,
            tc.tile_pool(name="ysb", bufs=2) as ypool,
            tc.tile_pool(name="blend", bufs=2) as bpool,
            tc.tile_pool(name="osb", bufs=3) as opool,
            tc.tile_pool(name="hps", bufs=3, space="PSUM") as hps_pool,
            tc.tile_pool(name="h2ps", bufs=3, space="PSUM") as h2ps_pool,
            tc.tile_pool(name="zps", bufs=1, space="PSUM") as zps_pool,
        ):
            # ---- load constants once ----
            w0t = cpool.tile([128, 128], BF16, tag="w0t")
            w1bd = cpool.tile([128, 128], BF16, tag="w1bd")
            g2 = cpool.tile([128, 24], BF16, tag="g2")
            nc.sync.dma_start(w0t[:], w0t_d[:])
            nc.sync.dma_start(w1bd[:], w1bd_d[:])
            nc.sync.dma_start(g2[:], g2_d[:])
            if b01_nonzero:
                b0c = cpool.tile([128, 1], F32, tag="b0c")
                b1c = cpool.tile([128, 1], F32, tag="b1c")
                nc.sync.dma_start(b0c[:], b0_d[:])
                nc.sync.dma_start(b1c[:], b1_d[:])
            if b2_nonzero:
                b2r = cpool.tile([1, 192], F32, tag="b2r")
                nc.sync.dma_start(b2r[:], b2_d[:])

            def relu_evac(dst, src, bias_tile, on_act):
                # relu is bit-exact on either engine; bias handling matches
                if on_act:
                    bias = bias_tile[:] if bias_tile is not None else 0.0
                    nc.scalar.activation(
                        dst, src, mybir.ActivationFunctionType.Relu, bias=bias)
                else:
                    s1 = bias_tile[:] if bias_tile is not None else 0.0
                    nc.vector.tensor_scalar(
                        out=dst, in0=src, scalar1=s1, scalar2=0.0,
                        op0=mybir.AluOpType.add, op1=mybir.AluOpType.max)

            for b in range(n_batch):
                # ---- loads (SWDGE cast f32->bf16, u8->bf16) ----
                x_bf = xpool.tile([128, PPP * IN_DIM], BF16, tag="x")   # [128,2048]
                nc.gpsimd.dma_start(x_bf[:], x_v[b])
                mask_bf = mpool.tile([128, PPP], BF16, tag="m")
                nc.gpsimd.dma_start(mask_bf[:], m_v[b])

                # ---- feature-major via one XBAR DMA-transpose (SBUF->SBUF,
                # 3D out = per-[128,128]-chunk transpose) ----
                t_sb = tpool.tile([128, PPP * IN_DIM], BF16, tag="t")
                nc.sync.dma_start(
                    t_sb[:].rearrange("p (c q) -> p c q", c=16),
                    x_bf[:],
                    transpose=True,
                )

                y_sb = ypool.tile([128, PPP * 3], BF16, tag="y")        # [128,768]
                z_ps = zps_pool.tile([128, 1024], F32, tag="z")
                for st in range(N_ST):
                    for s in range(2):
                        # ---- L0 (64x128 row-tiled, 2 concurrent tiles) ----
                        h_ps = hps_pool.tile([128, 512], F32, tag="h")
                        nc.tensor.matmul(
                            h_ps[:],
                            lhsT=w0t[s * 64:(s + 1) * 64, :],
                            rhs=t_sb[s * 64:(s + 1) * 64,
                                     st * 512:(st + 1) * 512],
                        )
                        h_sb = hpool.tile([128, 512], BF16, tag="hs")
                        relu_evac(h_sb[:], h_ps[:],
                                  b0c if b01_nonzero else None, True)

                        # ---- L1 (full array) ----
                        h2_ps = h2ps_pool.tile([128, 512], F32, tag="h2")
                        nc.tensor.matmul(h2_ps[:], lhsT=w1bd[:], rhs=h_sb[:])
                        h2_sb = h2pool.tile([128, 512], BF16, tag="h2s")
                        relu_evac(h2_sb[:], h2_ps[:],
                                  b1c if b01_nonzero else None,
                                  (st, s) in ACT_RELU2_UNITS)

                        # ---- L2 fused with out-transpose ----
                        # z[p, slot]: pixel p*PPP + st*64 + c*16 + s*8 + g,
                        # slot = (st*64 + c*16 + s*8 + g)*3 + ch
                        for c in range(4):
                            col = st * 256 + c * 48 + s * 24
                            nc.tensor.matmul(
                                z_ps[:, col:col + 24],
                                lhsT=h2_sb[:, c * 128:(c + 1) * 128],
                                rhs=g2[:],
                            )

                # optional +b2 (skipped when b2 == 0)
                z_view = z_ps[:].rearrange("p (st k) -> p st k", st=4)[:, :, :192]
                if b2_nonzero:
                    nc.vector.tensor_tensor(
                        out=z_view,
                        in0=z_view,
                        in1=b2r[:].partition_broadcast(128).rearrange(
                            "p k -> p 1 k").broadcast_to([128, 4, 192]),
                        op=mybir.AluOpType.add,
                    )

                # ---- sigmoid (one op per batch, strided PSUM view) ----
                # The 1-col touch absorbs the WAR wait on the recycled y_sb
                # slot (a DVE reader) so the sigmoid itself only waits on the
                # PE semaphore; the WAW on col 0 orders touch before sigmoid.
                nc.scalar.activation(
                    y_sb[:, 0:1], y_sb[:, 0:1],
                    mybir.ActivationFunctionType.Relu)
                nc.scalar.activation(
                    y_sb[:].rearrange("p (st k) -> p st k", st=4),
                    z_view,
                    mybir.ActivationFunctionType.Sigmoid,
                )

                # ---- blend: out = a*rgb + b*y,  b = mask*res, a = mask - b
                # (bit-exact with mask*(1-res): for mask==1, b==res exactly)
                x3 = x_bf[:].rearrange("p (n f) -> p n f", f=IN_DIM)
                rgb = x3[:, :, 0:3]
                res = x3[:, :, 3]
                bco = bpool.tile([128, PPP], BF16, tag="bc")
                aco = bpool.tile([128, PPP], BF16, tag="ac")
                # bco = res*mask is exact under any rounding mode (mask is
                # 0/1), so it is safe on GPSIMD despite its 1-ulp rounding
                # differences; aco = mask - bco needs RNE -> stays on DVE.
                nc.gpsimd.tensor_tensor(out=bco[:], in0=res, in1=mask_bf[:],
                                        op=mybir.AluOpType.mult)
                nc.vector.tensor_tensor(out=aco[:], in0=mask_bf[:], in1=bco[:],
                                        op=mybir.AluOpType.subtract)
                u_sb = bpool.tile([128, PPP * 3], BF16, tag="u")
                v_sb = bpool.tile([128, PPP * 3], BF16, tag="v")
                o_sb = opool.tile([128, PPP * 3], BF16, tag="o")
                u3 = u_sb[:].rearrange("p (n c) -> p n c", c=3)
                v3 = v_sb[:].rearrange("p (n c) -> p n c", c=3)
                y3 = y_sb[:].rearrange("p (n c) -> p n c", c=3)
                nc.vector.tensor_tensor(
                    out=u3, in0=rgb,
                    in1=aco[:].unsqueeze(2).broadcast_to([128, PPP, 3]),
                    op=mybir.AluOpType.mult)
                nc.vector.tensor_tensor(
                    out=v3, in0=y3,
                    in1=bco[:].unsqueeze(2).broadcast_to([128, PPP, 3]),
                    op=mybir.AluOpType.mult)
                nc.vector.tensor_tensor(out=o_sb[:], in0=v_sb[:], in1=u_sb[:],
                                        op=mybir.AluOpType.add)

                # ---- store (SWDGE cast bf16->f32) ----
                nc.gpsimd.dma_start(o_v[b], o_sb[:])

    nc.finalize()
    return nc


_PROGRAM_CACHE = {}


def _get_program(npc, b01_nonzero, b2_nonzero):
    key = (npc, b01_nonzero, b2_nonzero)
    if key not in _PROGRAM_CACHE:
        _PROGRAM_CACHE[key] = build_program(npc, b01_nonzero, b2_nonzero)
    return _PROGRAM_CACHE[key]


def _build_in_maps(x, mask, w0, b0, w1, b1, w2, b2):
    x = np.asarray(x, np.float32)
    mask_u8 = np.asarray(mask).astype(np.uint8)
    consts = _prep_weights(
        np.asarray(w0, np.float32), np.asarray(b0, np.float32),
        np.asarray(w1, np.float32), np.asarray(b1, np.float32),
        np.asarray(w2, np.float32), np.asarray(b2, np.float32))
    x_flat = np.ascontiguousarray(x.reshape(NPX, IN_DIM))
    m_flat = np.ascontiguousarray(mask_u8.reshape(NPX))
    nc = _get_program(NPC, consts["b01_nonzero"], consts["b2_nonzero"])
    const_map = {k: np.asarray(v) for k, v in consts.items()
                 if k not in ("b2_nonzero", "b01_nonzero")}
    in_maps = []
    for k in range(N_CORES):
        lo, hi = k * NPC, (k + 1) * NPC
        in_maps.append({"x": x_flat[lo:hi], "mask": m_flat[lo:hi], **const_map})
    return nc, in_maps


def kernel(x, mask, w0, b0, w1, b1, w2, b2):
    nc, in_maps = _build_in_maps(x, mask, w0, b0, w1, b1, w2, b2)
    res = run_bass_kernel_spmd(nc, in_maps, core_ids=list(range(N_CORES)))
    out = np.concatenate([res.results[k]["out"] for k in range(N_CORES)], axis=0)
    return out.reshape(B, H, W, OUT_DIM)


def run_traced(**inputs):
    """Run with NTFF tracing; returns the BassKernelResults (exec_time_ns)."""
    nc, in_maps = _build_in_maps(**inputs)
    return run_bass_kernel_spmd(
        nc, in_maps, core_ids=list(range(N_CORES)), trace=True,
        stitch_traces=False)


if __name__ == "__main__":
    # quick smoke test with random data
    rng = np.random.default_rng(0)
    x = rng.random((B, H, W, IN_DIM), np.float32)
    mask = rng.integers(0, 2, (B, H, W)).astype(bool)
    w0 = rng.standard_normal((IN_DIM, HID)).astype(np.float32) * 0.5
    b0 = np.zeros(HID, np.float32)
    w1 = rng.standard_normal((HID, HID)).astype(np.float32) * 0.3
    b1 = np.zeros(HID, np.float32)
    w2 = rng.standard_normal((HID, OUT_DIM)).astype(np.float32) * 0.3
    b2 = np.zeros(OUT_DIM, np.float32)
    out = kernel(x=x, mask=mask, w0=w0, b0=b0, w1=w1, b1=b1, w2=w2, b2=b2)
    print("out", out.shape, out.dtype, out[0, 0, :2])
